# revision 18
# baseline (speedup 1.0000x reference)
"""GRU-D cell (nn_GRUDCell), data-parallel Bass/Tile kernel for 8 TRN2 NeuronCores.

Strategy
--------
Shard the batch dim (16384 -> 8 x 2048) across cores; replicate the 512x512
weights.  Per core, the batch is processed in slices (256/512 rows),
software-pipelined as A(s) = load+elementwise+transpose, B(s) =
matmuls+gates+store, emitted A(0), A(1), B(0), A(2), B(1), ... so no engine
FIFO has head-of-line blocking between consecutive slices.  The first and
last slices are half-size (fast PE ramp-up, short drain tail), and only the
first gate's weights (W_r/U_r) load before A(0) so the startup DMAs stay off
the critical path.

 1. SWDGE cast-DMA loads (fp32 HBM -> bf16 SBUF), one merged DMA per slice.
 2. Element-wise imputation / decay in bf16, batch-major, batched to
    [128, sb] per op on DVE + ACT.  Gate sigmoids are computed as
    sigmoid(v) = 0.5 + 0.5*tanh(v/2) so a single ACT table set
    (`exp_and_others`: exp + tanh) serves the whole kernel.
 3. One DMA-xbar transpose per [128b, 512f] tile flips x_dec / h_dec into
    feature-major [512f, batch] layout for the matmuls.
 4. TensorE: six 512x512 gate matmuls in bf16, weights stationary (lhsT),
    accumulating X- and H- contributions into one PSUM bank per (gate, m).
    U_h is pre-scaled by 0.5 on-device so rh' = (1+tanh(r/2))*h_dec
    (= 2*r*h_dec) feeds the candidate matmul without extra scaling.
 5. ACT evacuates PSUM through tanh (+ per-partition bias), DVE finishes the
    convex combine, a reverse xbar transpose restores batch-major, and one
    merged SWDGE cast-DMA per slice stores fp32.

Host-side value-adaptive fast paths (checked against the actual call inputs,
with a fully general fallback): equal x/h decay vectors (share one gamma),
zero mean_imputation (drop the imputation-mean terms), zero biases (skip
bias loads).
"""

import numpy as np

import concourse.bacc as bacc
import concourse.mybir as mybir
from concourse.tile import TileContext
from concourse import bass_utils

F = 512               # feature dim == units
P = 128               # partitions
NM = F // P           # 4 feature chunks of 128
N_CORES = 8
B_TOTAL = 16384
BC = B_TOTAL // N_CORES  # 2048 rows per core

FP32 = mybir.dt.float32
BF16 = mybir.dt.bfloat16
AF = mybir.ActivationFunctionType
OP = mybir.AluOpType

_WEIGHT_KEYS = ("W_z", "U_z", "b_z", "W_r", "U_r", "b_r", "W_h", "U_h", "b_h",
                "gamma_x_decay", "gamma_h_decay", "mean_imputation")


def _slice_plan(bc, fast):
    if not fast:
        return [(i * 256, 256) for i in range(bc // 256)] if bc >= 256 else [(0, bc)]
    if bc < 1024:
        return [(i * 256, 256) for i in range(bc // 256)] if bc >= 256 else [(0, bc)]
    sbs = [256] + [512] * ((bc - 512) // 512) + [256]
    plan, r0 = [], 0
    for sb in sbs:
        plan.append((r0, sb))
        r0 += sb
    return plan


def _build(bc=BC, eq_decay=False, mu_zero=False, bias_zero=False):
    """Build + compile the per-core kernel for a batch shard of `bc` rows."""
    fast = eq_decay and mu_zero and bias_zero
    plan = _slice_plan(bc, fast)
    nslice = len(plan)
    max_nbt = max(sb for _, sb in plan) // P

    nc = bacc.Bacc("TRN2", target_bir_lowering=False, debug=False,
                   enable_asserts=False)

    inp = nc.dram_tensor("inputs", [bc, 3 * F], FP32, kind="ExternalInput").ap()
    hpv = nc.dram_tensor("h_prev", [bc, F], FP32, kind="ExternalInput").ap()
    wmats = {
        name: nc.dram_tensor(name, [F, F], FP32, kind="ExternalInput").ap()
        for name in ("W_r", "U_r", "W_z", "U_z", "W_h", "U_h")
    }
    vecs = {
        name: nc.dram_tensor(name, [F], FP32, kind="ExternalInput").ap()
        for name in ("b_z", "b_r", "b_h",
                     "gamma_x_decay", "gamma_h_decay", "mean_imputation")
    }
    out = nc.dram_tensor("out", [bc, F], FP32, kind="ExternalOutput").ap()

    with TileContext(nc) as tc:
        with (
            tc.tile_pool(name="const", bufs=1) as const,
            tc.tile_pool(name="stage", bufs=1) as stage,
            tc.tile_pool(name="raw", bufs=2) as rawp,
            tc.tile_pool(name="hp", bufs=2) as hpp,
            tc.tile_pool(name="tmp", bufs=6) as tmpp,
            tc.tile_pool(name="btmp", bufs=4) as btmpp,
            tc.tile_pool(name="ew", bufs=2) as ewp,
            tc.tile_pool(name="tpose", bufs=2) as tpp,
            tc.tile_pool(name="gates", bufs=2) as gp,
            tc.tile_pool(name="fin", bufs=2) as fin,
            tc.tile_pool(name="store", bufs=2) as stp,
            tc.tile_pool(name="ps", bufs=8, space="PSUM") as psp,
        ):
            st = [dict() for _ in range(nslice)]

            def emit_loads(s):
                r0, sb = plan[s]
                nbt = sb // P
                raw = rawp.tile([P, nbt, 3 * F], BF16, tag="raw")
                nc.gpsimd.dma_start(
                    raw[:], inp[r0:r0 + sb, :].rearrange("(bt p) f -> p bt f", p=P))
                hps = hpp.tile([P, nbt, F], BF16, tag="hp")
                nc.gpsimd.dma_start(
                    hps[:], hpv[r0:r0 + sb, :].rearrange("(bt p) f -> p bt f", p=P))
                st[s]["raw"], st[s]["hp"] = raw, hps

            # --- loads for the first two slices go before everything else
            emit_loads(0)
            if nslice > 1:
                emit_loads(1)

            # ---- per-feature rows (HWDGE load, broadcast, bf16-replicate);
            #      these feed the first DVE op, so they come before weights
            def repl_row(name, relu):
                row1 = stage.tile([1, F], FP32, tag=f"row1_{name}")
                nc.sync.dma_start(row1[:], vecs[name].unsqueeze(0))
                rowf = stage.tile([P, F], FP32, tag=f"rowf_{name}")
                nc.gpsimd.partition_broadcast(rowf[:], row1[:])
                wide = const.tile([P, max_nbt, F], BF16, tag=f"row_{name}")
                for j in range(max_nbt):
                    if relu:
                        nc.vector.tensor_scalar_max(wide[:, j, :], rowf[:], 0.0)
                    else:
                        nc.vector.tensor_copy(wide[:, j, :], rowf[:])
                return wide

            cx_w = repl_row("gamma_x_decay", relu=True)
            ch_w = None if eq_decay else repl_row("gamma_h_decay", relu=True)
            mu_w = None if mu_zero else repl_row("mean_imputation", relu=False)

            # ---- per-chunk biases [128, NM] fp32 (z/r pre-scaled by 0.5)
            if bias_zero:
                bz_h = br_h = bh_c = None
            else:
                def bias_chunks(name, half):
                    t = const.tile([P, NM], FP32, tag=f"b_{name}")
                    nc.sync.dma_start(t[:], vecs[name].rearrange("(m p) -> p m", p=P))
                    if half:
                        nc.vector.tensor_scalar_mul(t[:], t[:], 0.5)
                    return t

                bz_h = bias_chunks("b_z", half=True)
                br_h = bias_chunks("b_r", half=True)
                bh_c = bias_chunks("b_h", half=False)

            # ---- weights: one merged cast-DMA per matrix, bf16, layout
            #      [128 (p within k-chunk), k-chunk, 512 u].  W_r/U_r load
            #      now (first gate computed); the rest load after A(0).
            w_sb = {}

            def load_weight(name):
                t = const.tile([P, NM, F], BF16, tag=f"w_{name}")
                nc.gpsimd.dma_start(t[:], wmats[name].rearrange("(k p) u -> p k u", p=P))
                if name == "U_h":
                    nc.vector.tensor_scalar_mul(t[:], t[:], 0.5)
                w_sb[name] = t

            load_weight("W_r")
            load_weight("U_r")

            def lhsT(name, k, mi):
                return w_sb[name][:, k, mi * P:(mi + 1) * P]

            def bias_ap(bt, mi):
                return 0.0 if bt is None else bt[:, mi:mi + 1]

            def ew_chain(xdst, hdst, x_s, m_s, dt_s, hp_s, nb):
                """x_dec/h_dec element-wise chain on [P, nb*F] operands."""
                u1 = tmpp.tile([P, nb, F], BF16, tag="tmp")
                nc.vector.tensor_mul(u1[:], dt_s, cx_w[:, :nb, :])
                g1 = ewp.tile([P, nb, F], BF16, tag="g1")
                nc.scalar.activation(g1[:], u1[:], AF.Exp, scale=-1.0)
                if eq_decay:
                    g2 = g1
                else:
                    u2 = tmpp.tile([P, nb, F], BF16, tag="tmp")
                    nc.vector.tensor_mul(u2[:], dt_s, ch_w[:, :nb, :])
                    g2 = ewp.tile([P, nb, F], BF16, tag="g2")
                    nc.scalar.activation(g2[:], u2[:], AF.Exp, scale=-1.0)

                p1 = tmpp.tile([P, nb, F], BF16, tag="tmp")
                nc.vector.tensor_scalar(p1[:], m_s, -1.0, 1.0, OP.mult, OP.add)
                gm1 = tmpp.tile([P, nb, F], BF16, tag="tmp")
                nc.vector.tensor_scalar_sub(gm1[:], g1[:], 1.0)
                pq = tmpp.tile([P, nb, F], BF16, tag="tmp")
                nc.vector.tensor_mul(pq[:], gm1[:], p1[:])
                if mu_zero:
                    xq = tmpp.tile([P, nb, F], BF16, tag="tmp")
                    nc.vector.tensor_scalar_add(xq[:], pq[:], 1.0)
                    nc.vector.tensor_mul(xdst, x_s, xq[:])
                else:
                    t_ = tmpp.tile([P, nb, F], BF16, tag="tmp")
                    nc.vector.tensor_sub(t_[:], x_s, mu_w[:, :nb, :])
                    w_ = tmpp.tile([P, nb, F], BF16, tag="tmp")
                    nc.vector.tensor_mul(w_[:], pq[:], t_[:])
                    nc.vector.tensor_add(xdst, x_s, w_[:])
                nc.vector.tensor_mul(hdst, g2[:], hp_s)

            def emit_A(s):
                r0, sb = plan[s]
                nbt = sb // P
                xdT = tpp.tile([P, NM, sb], BF16, tag="xdT")
                hdT = tpp.tile([P, NM, sb], BF16, tag="hdT")
                raw, hps = st[s]["raw"], st[s]["hp"]
                xd = ewp.tile([P, nbt, F], BF16, tag="xd")
                hd = ewp.tile([P, nbt, F], BF16, tag="hd")
                ew_chain(xd[:], hd[:], raw[:, :, :F], raw[:, :, F:2 * F],
                         raw[:, :, 2 * F:], hps[:], nbt)
                for j in range(nbt):
                    nc.sync.dma_start(out=xdT[:, :, j * P:(j + 1) * P],
                                      in_=xd[:, j, :], transpose=True)
                    nc.sync.dma_start(out=hdT[:, :, j * P:(j + 1) * P],
                                      in_=hd[:, j, :], transpose=True)
                st[s]["xdT"], st[s]["hdT"] = xdT, hdT

            def emit_B(s):
                r0, sb = plan[s]
                nbt = sb // P
                xdT, hdT = st[s]["xdT"], st[s]["hdT"]
                # gates r and z: tanh(0.5*(X@W + H@U) + 0.5*b)
                tz_all = gp.tile([P, NM, sb], BF16, tag="tz")
                tr_all = gp.tile([P, NM, sb], BF16, tag="tr")
                for wname, uname, bias, dst in (
                        ("W_r", "U_r", br_h, tr_all),
                        ("W_z", "U_z", bz_h, tz_all)):
                    for mi in range(NM):
                        ps = psp.tile([P, sb], FP32, tag="ps")
                        for k in range(NM):
                            nc.tensor.matmul(ps[:], lhsT(wname, k, mi),
                                             xdT[:, k, :], start=(k == 0), stop=False)
                        for k in range(NM):
                            nc.tensor.matmul(ps[:], lhsT(uname, k, mi),
                                             hdT[:, k, :], start=False, stop=(k == NM - 1))
                        nc.scalar.activation(dst[:, mi, :], ps[:], AF.Tanh,
                                             bias=bias_ap(bias, mi), scale=0.5)

                # rh' = (tr+1)*h_dec (= 2*r*h_dec; U_h pre-scaled 0.5)
                trp = btmpp.tile([P, NM, sb], BF16, tag="btmp")
                nc.vector.tensor_scalar_add(trp[:], tr_all[:], 1.0)
                rh = fin.tile([P, NM, sb], BF16, tag="rh")
                nc.vector.tensor_mul(rh[:], trp[:], hdT[:])

                th_all = fin.tile([P, NM, sb], BF16, tag="th")
                for mi in range(NM):
                    ps = psp.tile([P, sb], FP32, tag="ps")
                    for k in range(NM):
                        nc.tensor.matmul(ps[:], lhsT("W_h", k, mi),
                                         xdT[:, k, :], start=(k == 0), stop=False)
                    for k in range(NM):
                        nc.tensor.matmul(ps[:], lhsT("U_h", k, mi),
                                         rh[:, k, :], start=False, stop=(k == NM - 1))
                    nc.scalar.activation(th_all[:, mi, :], ps[:], AF.Tanh,
                                         bias=bias_ap(bh_c, mi), scale=1.0)

                # h_new = h_dec + 0.5*(tz+1)*(h_hat - h_dec)
                d_all = btmpp.tile([P, NM, sb], BF16, tag="btmp")
                nc.vector.tensor_sub(d_all[:], th_all[:], hdT[:])
                tzp = btmpp.tile([P, NM, sb], BF16, tag="btmp")
                nc.vector.tensor_scalar(tzp[:], tz_all[:], 1.0, 0.5, OP.add, OP.mult)
                e_all = btmpp.tile([P, NM, sb], BF16, tag="btmp")
                nc.vector.tensor_mul(e_all[:], tzp[:], d_all[:])
                hn_all = fin.tile([P, NM, sb], BF16, tag="hn")
                nc.vector.tensor_add(hn_all[:], e_all[:], hdT[:])

                stT = stp.tile([P, nbt, F], BF16, tag="stT")
                for mi in range(NM):
                    nc.sync.dma_start(out=stT[:, :, mi * P:(mi + 1) * P],
                                      in_=hn_all[:, mi, :], transpose=True)

                nc.gpsimd.dma_start(
                    out[r0:r0 + sb, :].rearrange("(bt p) f -> p bt f", p=P), stT[:])

            # emission: l0, l1, (setup), A0, rest-of-weights,
            #           [A1, l2, B0], [A2, l3, B1], ...
            emit_A(0)
            for name in ("W_z", "U_z", "W_h", "U_h"):
                load_weight(name)
            for s in range(1, nslice):
                if s + 1 < nslice:
                    emit_loads(s + 1)
                emit_A(s)
                emit_B(s - 1)
            emit_B(nslice - 1)

    nc.compile()
    return nc


_NC_CACHE = {}


def _get_nc(flags=(False, False, False)):
    if flags not in _NC_CACHE:
        _NC_CACHE[flags] = _build(eq_decay=flags[0], mu_zero=flags[1],
                                  bias_zero=flags[2])
    return _NC_CACHE[flags]


def _flags_for(inputs):
    eq = bool(np.array_equal(np.asarray(inputs["gamma_x_decay"]),
                             np.asarray(inputs["gamma_h_decay"])))
    mu0 = not np.any(np.asarray(inputs["mean_imputation"]))
    b0 = not (np.any(np.asarray(inputs["b_z"])) or
              np.any(np.asarray(inputs["b_r"])) or
              np.any(np.asarray(inputs["b_h"])))
    return (eq, mu0, b0)


def kernel(**inputs) -> np.ndarray:
    nc = _get_nc(_flags_for(inputs))
    inp = np.ascontiguousarray(inputs["inputs"], dtype=np.float32)
    hp = np.ascontiguousarray(inputs["h_prev"], dtype=np.float32)
    b = inp.shape[0]
    inp = inp.reshape(N_CORES, b // N_CORES, 3 * F)
    hp = hp.reshape(N_CORES, b // N_CORES, F)
    shared = {k: np.ascontiguousarray(inputs[k], dtype=np.float32)
              for k in _WEIGHT_KEYS}
    in_maps = [dict(inputs=inp[c], h_prev=hp[c], **shared)
               for c in range(N_CORES)]
    res = bass_utils.run_bass_kernel_spmd(nc, in_maps,
                                          core_ids=list(range(N_CORES)))
    outs = [r["out"] for r in res.results]
    return np.concatenate(outs, axis=0).astype(np.float32)


# revision 20
# speedup vs baseline: 50498.0467x; 50498.0467x over previous
"""GRU-D cell (nn_GRUDCell), data-parallel Bass/Tile kernel for 8 TRN2 NeuronCores.

Strategy
--------
Shard the batch dim (16384 -> 8 x 2048) across cores; replicate the 512x512
weights.  Per core, the batch is processed in slices (256/512 rows),
software-pipelined as A(s) = load+elementwise+transpose, B(s) =
matmuls+gates+store, emitted A(0), A(1), B(0), A(2), B(1), ... so no engine
FIFO has head-of-line blocking between consecutive slices.  The first and
last slices are half-size (fast PE ramp-up, short drain tail), and only the
first gate's weights (W_r/U_r) load before A(0) so the startup DMAs stay off
the critical path.

 1. SWDGE cast-DMA loads (fp32 HBM -> bf16 SBUF), one merged DMA per slice.
 2. Element-wise imputation / decay in bf16, batch-major, batched to
    [128, sb] per op on DVE + ACT.  Gate sigmoids are computed as
    sigmoid(v) = 0.5 + 0.5*tanh(v/2) so a single ACT table set
    (`exp_and_others`: exp + tanh) serves the whole kernel.
 3. One DMA-xbar transpose per [128b, 512f] tile flips x_dec / h_dec into
    feature-major [512f, batch] layout for the matmuls.
 4. TensorE: six 512x512 gate matmuls in bf16, weights stationary (lhsT),
    accumulating X- and H- contributions into one PSUM bank per (gate, m).
    U_h is pre-scaled by 0.5 on-device so rh' = (1+tanh(r/2))*h_dec
    (= 2*r*h_dec) feeds the candidate matmul without extra scaling.
 5. ACT evacuates PSUM through tanh (+ per-partition bias), DVE finishes the
    convex combine, a reverse xbar transpose restores batch-major, and one
    merged SWDGE cast-DMA per slice stores fp32.

Host-side value-adaptive fast paths (checked against the actual call inputs,
with a fully general fallback): equal x/h decay vectors (share one gamma),
zero mean_imputation (drop the imputation-mean terms), zero biases (skip
bias loads).
"""

import numpy as np

import concourse.bacc as bacc
import concourse.mybir as mybir
from concourse.tile import TileContext
from concourse import bass_utils

F = 512               # feature dim == units
P = 128               # partitions
NM = F // P           # 4 feature chunks of 128
N_CORES = 8
B_TOTAL = 16384
BC = B_TOTAL // N_CORES  # 2048 rows per core

FP32 = mybir.dt.float32
BF16 = mybir.dt.bfloat16
AF = mybir.ActivationFunctionType
OP = mybir.AluOpType

_WEIGHT_KEYS = ("W_z", "U_z", "b_z", "W_r", "U_r", "b_r", "W_h", "U_h", "b_h",
                "gamma_x_decay", "gamma_h_decay", "mean_imputation")


def _slice_plan(bc, fast):
    if not fast:
        return [(i * 256, 256) for i in range(bc // 256)] if bc >= 256 else [(0, bc)]
    if bc < 1024:
        return [(i * 256, 256) for i in range(bc // 256)] if bc >= 256 else [(0, bc)]
    sbs = [256] + [512] * ((bc - 512) // 512) + [256]
    plan, r0 = [], 0
    for sb in sbs:
        plan.append((r0, sb))
        r0 += sb
    return plan


def _build(bc=BC, eq_decay=False, mu_zero=False, bias_zero=False):
    """Build + compile the per-core kernel for a batch shard of `bc` rows."""
    fast = eq_decay and mu_zero and bias_zero
    plan = _slice_plan(bc, fast)
    nslice = len(plan)
    max_nbt = max(sb for _, sb in plan) // P

    nc = bacc.Bacc("TRN2", target_bir_lowering=False, debug=False,
                   enable_asserts=False)

    inp = nc.dram_tensor("inputs", [bc, 3 * F], FP32, kind="ExternalInput").ap()
    hpv = nc.dram_tensor("h_prev", [bc, F], FP32, kind="ExternalInput").ap()
    wmats = {
        name: nc.dram_tensor(name, [F, F], FP32, kind="ExternalInput").ap()
        for name in ("W_r", "U_r", "W_z", "U_z", "W_h", "U_h")
    }
    vecs = {
        name: nc.dram_tensor(name, [F], FP32, kind="ExternalInput").ap()
        for name in ("b_z", "b_r", "b_h",
                     "gamma_x_decay", "gamma_h_decay", "mean_imputation")
    }
    out = nc.dram_tensor("out", [bc, F], FP32, kind="ExternalOutput").ap()

    with TileContext(nc) as tc:
        with (
            tc.tile_pool(name="const", bufs=1) as const,
            tc.tile_pool(name="stage", bufs=1) as stage,
            tc.tile_pool(name="raw", bufs=2) as rawp,
            tc.tile_pool(name="hp", bufs=2) as hpp,
            tc.tile_pool(name="tmp", bufs=6) as tmpp,
            tc.tile_pool(name="btmp", bufs=4) as btmpp,
            tc.tile_pool(name="ew", bufs=2) as ewp,
            tc.tile_pool(name="tpose", bufs=2) as tpp,
            tc.tile_pool(name="gates", bufs=2) as gp,
            tc.tile_pool(name="fin", bufs=2) as fin,
            tc.tile_pool(name="store", bufs=2) as stp,
            tc.tile_pool(name="ps", bufs=8, space="PSUM") as psp,
        ):
            st = [dict() for _ in range(nslice)]

            def emit_loads(s):
                r0, sb = plan[s]
                nbt = sb // P
                raw = rawp.tile([P, nbt, 3 * F], BF16, tag="raw")
                nc.gpsimd.dma_start(
                    raw[:], inp[r0:r0 + sb, :].rearrange("(bt p) f -> p bt f", p=P))
                hps = hpp.tile([P, nbt, F], BF16, tag="hp")
                nc.gpsimd.dma_start(
                    hps[:], hpv[r0:r0 + sb, :].rearrange("(bt p) f -> p bt f", p=P))
                st[s]["raw"], st[s]["hp"] = raw, hps

            # --- loads for the first two slices go before everything else
            emit_loads(0)
            if nslice > 1:
                emit_loads(1)

            # ---- per-feature rows (HWDGE load, broadcast, bf16-replicate);
            #      these feed the first DVE op, so they come before weights
            def repl_row(name, relu):
                row1 = stage.tile([1, F], FP32, tag=f"row1_{name}")
                nc.sync.dma_start(row1[:], vecs[name].unsqueeze(0))
                rowf = stage.tile([P, F], FP32, tag=f"rowf_{name}")
                nc.gpsimd.partition_broadcast(rowf[:], row1[:])
                wide = const.tile([P, max_nbt, F], BF16, tag=f"row_{name}")
                for j in range(max_nbt):
                    if relu:
                        nc.vector.tensor_scalar_max(wide[:, j, :], rowf[:], 0.0)
                    else:
                        nc.vector.tensor_copy(wide[:, j, :], rowf[:])
                return wide

            cx_w = repl_row("gamma_x_decay", relu=True)
            ch_w = None if eq_decay else repl_row("gamma_h_decay", relu=True)
            mu_w = None if mu_zero else repl_row("mean_imputation", relu=False)

            # ---- per-chunk biases [128, NM] fp32 (z/r pre-scaled by 0.5)
            if bias_zero:
                bz_h = br_h = bh_c = None
            else:
                def bias_chunks(name, half):
                    t = const.tile([P, NM], FP32, tag=f"b_{name}")
                    nc.sync.dma_start(t[:], vecs[name].rearrange("(m p) -> p m", p=P))
                    if half:
                        nc.vector.tensor_scalar_mul(t[:], t[:], 0.5)
                    return t

                bz_h = bias_chunks("b_z", half=True)
                br_h = bias_chunks("b_r", half=True)
                bh_c = bias_chunks("b_h", half=False)

            # ---- weights: one merged cast-DMA per matrix, bf16, layout
            #      [128 (p within k-chunk), k-chunk, 512 u].  W_r/U_r load
            #      now (first gate computed); the rest load after A(0).
            w_sb = {}

            def load_weight(name):
                t = const.tile([P, NM, F], BF16, tag=f"w_{name}")
                nc.gpsimd.dma_start(t[:], wmats[name].rearrange("(k p) u -> p k u", p=P))
                if name == "U_h":
                    nc.vector.tensor_scalar_mul(t[:], t[:], 0.5)
                w_sb[name] = t

            load_weight("W_r")
            load_weight("U_r")

            def lhsT(name, k, mi):
                return w_sb[name][:, k, mi * P:(mi + 1) * P]

            def bias_ap(bt, mi):
                return 0.0 if bt is None else bt[:, mi:mi + 1]

            def ew_chain(xdst, hdst, x_s, m_s, dt_s, hp_s, nb):
                """x_dec/h_dec element-wise chain on [P, nb*F] operands."""
                u1 = tmpp.tile([P, nb, F], BF16, tag="tmp")
                nc.vector.tensor_mul(u1[:], dt_s, cx_w[:, :nb, :])
                g1 = ewp.tile([P, nb, F], BF16, tag="g1")
                nc.scalar.activation(g1[:], u1[:], AF.Exp, scale=-1.0)
                if eq_decay:
                    g2 = g1
                else:
                    u2 = tmpp.tile([P, nb, F], BF16, tag="tmp")
                    nc.vector.tensor_mul(u2[:], dt_s, ch_w[:, :nb, :])
                    g2 = ewp.tile([P, nb, F], BF16, tag="g2")
                    nc.scalar.activation(g2[:], u2[:], AF.Exp, scale=-1.0)

                p1 = tmpp.tile([P, nb, F], BF16, tag="tmp")
                nc.vector.tensor_scalar(p1[:], m_s, -1.0, 1.0, OP.mult, OP.add)
                gm1 = tmpp.tile([P, nb, F], BF16, tag="tmp")
                nc.vector.tensor_scalar_sub(gm1[:], g1[:], 1.0)
                pq = tmpp.tile([P, nb, F], BF16, tag="tmp")
                nc.vector.tensor_mul(pq[:], gm1[:], p1[:])
                if mu_zero:
                    xq = tmpp.tile([P, nb, F], BF16, tag="tmp")
                    nc.vector.tensor_scalar_add(xq[:], pq[:], 1.0)
                    nc.vector.tensor_mul(xdst, x_s, xq[:])
                else:
                    t_ = tmpp.tile([P, nb, F], BF16, tag="tmp")
                    nc.vector.tensor_sub(t_[:], x_s, mu_w[:, :nb, :])
                    w_ = tmpp.tile([P, nb, F], BF16, tag="tmp")
                    nc.vector.tensor_mul(w_[:], pq[:], t_[:])
                    nc.vector.tensor_add(xdst, x_s, w_[:])
                nc.vector.tensor_mul(hdst, g2[:], hp_s)

            def emit_A(s):
                r0, sb = plan[s]
                nbt = sb // P
                xdT = tpp.tile([P, NM, sb], BF16, tag="xdT")
                hdT = tpp.tile([P, NM, sb], BF16, tag="hdT")
                raw, hps = st[s]["raw"], st[s]["hp"]
                xd = ewp.tile([P, nbt, F], BF16, tag="xd")
                hd = ewp.tile([P, nbt, F], BF16, tag="hd")
                ew_chain(xd[:], hd[:], raw[:, :, :F], raw[:, :, F:2 * F],
                         raw[:, :, 2 * F:], hps[:], nbt)
                for j in range(nbt):
                    nc.sync.dma_start(out=xdT[:, :, j * P:(j + 1) * P],
                                      in_=xd[:, j, :], transpose=True)
                    nc.sync.dma_start(out=hdT[:, :, j * P:(j + 1) * P],
                                      in_=hd[:, j, :], transpose=True)
                st[s]["xdT"], st[s]["hdT"] = xdT, hdT

            def emit_B(s):
                r0, sb = plan[s]
                nbt = sb // P
                xdT, hdT = st[s]["xdT"], st[s]["hdT"]
                # gates r and z: tanh(0.5*(X@W + H@U) + 0.5*b)
                tz_all = gp.tile([P, NM, sb], BF16, tag="tz")
                tr_all = gp.tile([P, NM, sb], BF16, tag="tr")
                for wname, uname, bias, dst in (
                        ("W_r", "U_r", br_h, tr_all),
                        ("W_z", "U_z", bz_h, tz_all)):
                    for mi in range(NM):
                        ps = psp.tile([P, sb], FP32, tag="ps")
                        for k in range(NM):
                            nc.tensor.matmul(ps[:], lhsT(wname, k, mi),
                                             xdT[:, k, :], start=(k == 0), stop=False)
                        for k in range(NM):
                            nc.tensor.matmul(ps[:], lhsT(uname, k, mi),
                                             hdT[:, k, :], start=False, stop=(k == NM - 1))
                        nc.scalar.activation(dst[:, mi, :], ps[:], AF.Tanh,
                                             bias=bias_ap(bias, mi), scale=0.5)

                # rh' = (tr+1)*h_dec (= 2*r*h_dec; U_h pre-scaled 0.5)
                trp = btmpp.tile([P, NM, sb], BF16, tag="btmp")
                nc.vector.tensor_scalar_add(trp[:], tr_all[:], 1.0)
                rh = fin.tile([P, NM, sb], BF16, tag="rh")
                nc.vector.tensor_mul(rh[:], trp[:], hdT[:])

                th_all = fin.tile([P, NM, sb], BF16, tag="th")
                for mi in range(NM):
                    ps = psp.tile([P, sb], FP32, tag="ps")
                    for k in range(NM):
                        nc.tensor.matmul(ps[:], lhsT("W_h", k, mi),
                                         xdT[:, k, :], start=(k == 0), stop=False)
                    for k in range(NM):
                        nc.tensor.matmul(ps[:], lhsT("U_h", k, mi),
                                         rh[:, k, :], start=False, stop=(k == NM - 1))
                    nc.scalar.activation(th_all[:, mi, :], ps[:], AF.Tanh,
                                         bias=bias_ap(bh_c, mi), scale=1.0)

                # h_new = h_dec + 0.5*(tz+1)*(h_hat - h_dec)
                d_all = btmpp.tile([P, NM, sb], BF16, tag="btmp")
                nc.vector.tensor_sub(d_all[:], th_all[:], hdT[:])
                tzp = btmpp.tile([P, NM, sb], BF16, tag="btmp")
                nc.vector.tensor_scalar(tzp[:], tz_all[:], 1.0, 0.5, OP.add, OP.mult)
                e_all = btmpp.tile([P, NM, sb], BF16, tag="btmp")
                nc.vector.tensor_mul(e_all[:], tzp[:], d_all[:])
                hn_all = fin.tile([P, NM, sb], BF16, tag="hn")
                nc.vector.tensor_add(hn_all[:], e_all[:], hdT[:])

                stT = stp.tile([P, nbt, F], BF16, tag="stT")
                for mi in range(NM):
                    nc.sync.dma_start(out=stT[:, :, mi * P:(mi + 1) * P],
                                      in_=hn_all[:, mi, :], transpose=True)

                nc.gpsimd.dma_start(
                    out[r0:r0 + sb, :].rearrange("(bt p) f -> p bt f", p=P), stT[:])

            # emission: l0, l1, (setup), A0, rest-of-weights,
            #           [A1, l2, B0], [A2, l3, B1], ...
            emit_A(0)
            for name in ("W_z", "U_z", "W_h", "U_h"):
                load_weight(name)
            for s in range(1, nslice):
                if s + 1 < nslice:
                    emit_loads(s + 1)
                emit_A(s)
                emit_B(s - 1)
            emit_B(nslice - 1)

    nc.compile()
    return nc


_NC_CACHE = {}


def _get_nc(flags=(False, False, False)):
    if flags not in _NC_CACHE:
        _NC_CACHE[flags] = _build(eq_decay=flags[0], mu_zero=flags[1],
                                  bias_zero=flags[2])
    return _NC_CACHE[flags]


def _flags_for(inputs):
    eq = bool(np.array_equal(np.asarray(inputs["gamma_x_decay"]),
                             np.asarray(inputs["gamma_h_decay"])))
    mu0 = not np.any(np.asarray(inputs["mean_imputation"]))
    b0 = not (np.any(np.asarray(inputs["b_z"])) or
              np.any(np.asarray(inputs["b_r"])) or
              np.any(np.asarray(inputs["b_h"])))
    return (eq, mu0, b0)


def kernel(**inputs) -> np.ndarray:
    nc = _get_nc(_flags_for(inputs))
    inp = np.ascontiguousarray(inputs["inputs"], dtype=np.float32)
    hp = np.ascontiguousarray(inputs["h_prev"], dtype=np.float32)
    b = inp.shape[0]
    inp = inp.reshape(N_CORES, b // N_CORES, 3 * F)
    hp = hp.reshape(N_CORES, b // N_CORES, F)
    shared = {k: np.ascontiguousarray(inputs[k], dtype=np.float32)
              for k in _WEIGHT_KEYS}
    in_maps = [dict(inputs=inp[c], h_prev=hp[c], **shared)
               for c in range(N_CORES)]
    res = bass_utils.run_bass_kernel_spmd(nc, in_maps,
                                          core_ids=list(range(N_CORES)))
    outs = [r["out"] for r in res.results]
    return np.concatenate(outs, axis=0).astype(np.float32)
